# revision 1
# baseline (speedup 1.0000x reference)
"""2-layer LSTM decoder for trn2 — 2-core tensor-parallel over the hidden dim.

The execution backend charges roughly per *instruction* (axon/fake_nrt
path), so this kernel minimizes total instruction count across cores:

- 2 cores only: core c owns hidden dims [512c, 512c+512) of every gate.
  All matmuls are full-width (N=512 batch moving operand, 128x128
  stationary), which is the per-MAC-cheapest shape. Global MM count per
  step is the structural floor (25 K-chunks x 32 gate tiles).
- One AllGather per layer per step exchanges the 512-wide h halves
  (4 instrs per layer per core total, incl. staging DMAs).
- x inputs (prev_y/known/gv/ones+bias row) are preassembled on the host
  into 8-step blocks -> one DMA per 8 steps. Predictions accumulate in
  an SBUF row and are written out once per 8 steps.
- L0 bias rides the x-chunk as a K-row against the constant ones row;
  L1 bias rides the per-(gate,chunk) activation's per-partition bias.
- Elementwise cell ops run once per layer on [128, 4, 512] tiles.
"""
import numpy as np
import ml_dtypes

import concourse.bass as bass
import concourse.mybir as mybir
import concourse.tile as tile
from concourse import bacc

F32 = mybir.dt.float32
BF16 = mybir.dt.bfloat16
AF = mybir.ActivationFunctionType
ALU = mybir.AluOpType

B, T_FULL, F, H, GE = 512, 168, 32, 1024, 16
N_CORES = 2                # cores actually used
NJ = 8                     # global hidden chunks (H/128)
LJ = 4                     # local hidden chunks per core
KX = 50                    # x-chunk rows: prev(1) + known(32) + gv(16) + ones(1)
GC = 2048                  # gate cols per core


def prep_host(inputs, T):
    inp = {k: np.asarray(v) for k, v in inputs.items()}
    gv_all = inp["group_emb"][inp["group_ids"].astype(np.int64)]   # (B, GE)
    b0 = (inp["b_ih0"] + inp["b_hh0"]).astype(np.float32)          # (4096,)
    b1 = (inp["b_ih1"] + inp["b_hh1"]).astype(np.float32)

    def core_cols(w_g, c):
        """(K, 4096) global gate cols -> (K, 2048) cols owned by core c,
        tiled m = X*4+j major (X = gate type, j = local hidden chunk)."""
        K = w_g.shape[0]
        a = w_g.reshape(K, 4, 8, 128)[:, :, 4 * c:4 * c + LJ, :]
        return np.ascontiguousarray(a.reshape(K, GC))

    # x-chunk (global cols): rows 0..48 = W_ih0.T, row 49 = b0
    w0x_g = np.zeros((128, 4096), np.float32)
    w0x_g[0:49] = inp["W_ih0"].astype(np.float32).T
    w0x_g[49] = b0
    whh0T = inp["W_hh0"].astype(np.float32).T                      # (1024, 4096)
    whh1T = inp["W_hh1"].astype(np.float32).T
    wih1T = inp["W_ih1"].astype(np.float32).T

    wp = inp["W_proj"].astype(np.float32)[0]                       # (1024,)
    wpT = np.ascontiguousarray(wp.reshape(8, 128).T).astype(ml_dtypes.bfloat16)

    NB8 = (T + 7) // 8
    knb = np.zeros((NB8, KX, 8, B), np.float32)
    kn = inp["dec_known"].astype(np.float32)                       # (B, T, F)
    y = inp["target_y"].astype(np.float32)[:, :, 0]                # (B, T)
    for t in range(T):
        b8, s = divmod(t, 8)
        knb[b8, 0, s] = (inp["last_enc_consumption"].astype(np.float32)[:, 0]
                         if t == 0 else y[:, t - 1])
        knb[b8, 1:33, s] = kn[:, t, :].T
        knb[b8, 33:49, s] = gv_all.T
        knb[b8, 49, s] = 1.0
    knb = knb.astype(ml_dtypes.bfloat16)

    shared = dict(wpT=wpT, knb=knb,
                  h0i=np.ascontiguousarray(
                      inp["h0"][0].astype(np.float32).reshape(B, 8, 128)
                      .transpose(2, 1, 0)).astype(ml_dtypes.bfloat16),
                  h1i=np.ascontiguousarray(
                      inp["h0"][1].astype(np.float32).reshape(B, 8, 128)
                      .transpose(2, 1, 0)).astype(ml_dtypes.bfloat16))
    per_core = []
    for c in range(N_CORES):
        w0 = np.zeros((128, 9 * GC), np.float32)
        w0[:, 0:GC] = core_cols(w0x_g, c)
        for k in range(NJ):
            w0[:, (k + 1) * GC:(k + 2) * GC] = core_cols(
                whh0T[128 * k:128 * (k + 1)], c)
        w1 = np.zeros((128, 16 * GC), np.float32)
        for k in range(NJ):
            w1[:, k * GC:(k + 1) * GC] = core_cols(
                whh1T[128 * k:128 * (k + 1)], c)
            w1[:, (8 + k) * GC:(9 + k) * GC] = core_cols(
                wih1T[128 * k:128 * (k + 1)], c)
        b1s = np.ascontiguousarray(
            b1.reshape(4, 8, 128)[:, 4 * c:4 * c + LJ, :]
            .transpose(2, 0, 1).reshape(128, 16)).astype(np.float32)
        sl = np.s_[:, 128 * 4 * c + np.arange(4 * 128)]
        d = dict(
            w0=w0.astype(ml_dtypes.bfloat16),
            w1=w1.astype(ml_dtypes.bfloat16),
            b1s=b1s,
            c0i=np.ascontiguousarray(
                inp["c0"][0].astype(np.float32)[:, 512 * c:512 * (c + 1)]
                .reshape(B, LJ, 128).transpose(2, 1, 0)).astype(np.float32),
            c1i=np.ascontiguousarray(
                inp["c0"][1].astype(np.float32)[:, 512 * c:512 * (c + 1)]
                .reshape(B, LJ, 128).transpose(2, 1, 0)).astype(np.float32),
        )
        per_core.append(d)
    tf_mask = [int(v) for v in np.asarray(inp["tf_mask"]).reshape(-1)][:T]
    b_proj = float(np.asarray(inp["b_proj"]).reshape(-1)[0])
    return shared, per_core, tf_mask, b_proj


def build_module(T, tf_mask, b_proj, rep=1):
    nc = bacc.Bacc(target_bir_lowering=False)
    NB8 = (T + 7) // 8

    w0_d = nc.dram_tensor("w0", [128, 9 * GC], BF16, kind="ExternalInput")
    w1_d = nc.dram_tensor("w1", [128, 16 * GC], BF16, kind="ExternalInput")
    b1s_d = nc.dram_tensor("b1s", [128, 16], F32, kind="ExternalInput")
    wpT_d = nc.dram_tensor("wpT", [128, 8], BF16, kind="ExternalInput")
    knb_d = nc.dram_tensor("knb", [NB8, KX, 8, B], BF16, kind="ExternalInput")
    h0i_d = nc.dram_tensor("h0i", [128, NJ, B], BF16, kind="ExternalInput")
    h1i_d = nc.dram_tensor("h1i", [128, NJ, B], BF16, kind="ExternalInput")
    c0i_d = nc.dram_tensor("c0i", [128, LJ, B], F32, kind="ExternalInput")
    c1i_d = nc.dram_tensor("c1i", [128, LJ, B], F32, kind="ExternalInput")
    out_d = nc.dram_tensor("out", [NB8, 8 * B], BF16, kind="ExternalOutput")

    RG = [[0, 1]]
    AFS = [AF.Sigmoid, AF.Sigmoid, AF.Tanh, AF.Sigmoid]   # i, f, g, o
    B1_BCAST = [True]   # disabled on first failure of broadcast_to

    with tile.TileContext(nc) as tc:
        with tc.tile_pool(name="const", bufs=1) as const, \
             tc.tile_pool(name="hfp", bufs=1) as hfp, \
             tc.tile_pool(name="act", bufs=1) as actp, \
             tc.tile_pool(name="st", bufs=1) as stp, \
             tc.tile_pool(name="xkp", bufs=2) as xkp, \
             tc.tile_pool(name="gps", bufs=2, space="PSUM") as gpsum, \
             tc.tile_pool(name="dram", bufs=2, space="DRAM") as dramp:

            w0_sb = const.tile([128, 9 * GC], BF16)
            nc.sync.dma_start(out=w0_sb[:], in_=w0_d[:])
            w1_sb = const.tile([128, 16 * GC], BF16)
            nc.sync.dma_start(out=w1_sb[:], in_=w1_d[:])
            b1s_sb = const.tile([128, 16], F32)
            nc.sync.dma_start(out=b1s_sb[:], in_=b1s_d[:])
            wpT_sb = const.tile([128, 8], BF16)
            nc.sync.dma_start(out=wpT_sb[:], in_=wpT_d[:])

            def w0_sl(k, X, j):
                base = k * GC + (X * LJ + j) * 128
                return w0_sb[:, base:base + 128]

            def w1_sl(k, X, j):
                base = k * GC + (X * LJ + j) * 128
                return w1_sb[:, base:base + 128]

            outbuf = const.tile([1, 8 * B], BF16)

            def rep_body(_rep):
                h0f = hfp.tile([128, NJ, B], BF16, tag="h0f", name=f"h0_{_rep}")
                nc.sync.dma_start(out=h0f[:], in_=h0i_d[:])
                h1f = hfp.tile([128, NJ, B], BF16, tag="h1f", name=f"h1_{_rep}")
                nc.sync.dma_start(out=h1f[:], in_=h1i_d[:])
                c0 = stp.tile([128, LJ, B], F32, tag="c0", name=f"c0_{_rep}")
                nc.sync.dma_start(out=c0[:], in_=c0i_d[:])
                c1 = stp.tile([128, LJ, B], F32, tag="c1", name=f"c1_{_rep}")
                nc.sync.dma_start(out=c1[:], in_=c1i_d[:])

                def emit_pred(t):
                    """pred(t) from h1f into outbuf slot t%8 (bf16)."""
                    pp = gpsum.tile([1, B], F32, tag="g", name=f"pp_{_rep}_{t}")
                    for k in range(NJ):
                        nc.tensor.matmul(pp[:], wpT_sb[:, k:k + 1],
                                         h1f[:, k, :],
                                         start=(k == 0), stop=(k == NJ - 1))
                    s = t % 8
                    nc.vector.tensor_scalar_add(
                        outbuf[0:1, s * B:(s + 1) * B], pp[:], b_proj)

                def cell(gsig, c_cur, hhalf_tag, t, lab):
                    """sig tiles (i,f,g,o as [128,LJ,B]) -> h half + c update."""
                    si, sf, sg, so = gsig
                    tmpf = actp.tile([128, LJ, B], F32, tag="tmpf",
                                     name=f"tf_{lab}_{t}")
                    nc.vector.tensor_tensor(out=tmpf[:], in0=sf[:], in1=c_cur[:],
                                            op=ALU.mult)
                    tmpb = actp.tile([128, LJ, B], BF16, tag="tt",
                                     name=f"tb_{lab}_{t}")
                    nc.vector.tensor_tensor(out=tmpb[:], in0=si[:], in1=sg[:],
                                            op=ALU.mult)
                    nc.vector.tensor_tensor(out=c_cur[:], in0=tmpf[:],
                                            in1=tmpb[:], op=ALU.add)
                    tanc = actp.tile([128, LJ, B], BF16, tag="tt",
                                     name=f"tc_{lab}_{t}")
                    nc.scalar.activation(tanc[:], c_cur[:], AF.Tanh)
                    hh = actp.tile([128, LJ, B], BF16, tag="hh",
                                   name=f"hh_{lab}_{t}")
                    nc.vector.tensor_tensor(out=hh[:], in0=so[:], in1=tanc[:],
                                            op=ALU.mult)
                    return hh

                def gather(hh, hf, tag, t):
                    cin = dramp.tile([128, LJ, B], BF16, tag=f"ci{tag}",
                                     name=f"ci{tag}_{t}")
                    cout = dramp.tile([2, 128, LJ, B], BF16, tag=f"co{tag}",
                                      name=f"co{tag}_{t}")
                    nc.sync.dma_start(out=cin[:], in_=hh[:])
                    nc.gpsimd.collective_compute(
                        "AllGather", ALU.bypass, ins=[cin[:]], outs=[cout[:]],
                        replica_groups=RG)
                    # one DMA: (src, p, j, b) -> (p, src*4+j, b)
                    nc.sync.dma_start(
                        out=hf[:].rearrange("p (s j) b -> p s j b", s=2),
                        in_=cout[:].transpose([1, 0, 2, 3]))

                xk = None
                for t in range(T):
                    b8, s = divmod(t, 8)
                    if s == 0:
                        if t > 0:
                            emit_pred(t - 1)
                            nc.sync.dma_start(out=out_d[b8 - 1:b8, :],
                                              in_=outbuf[:])
                        xk = xkp.tile([KX, 8, B], BF16, tag="xk",
                                      name=f"xk_{_rep}_{b8}")
                        nc.sync.dma_start(out=xk[:], in_=knb_d[b8])
                    elif t > 0:
                        emit_pred(t - 1)
                    if t > 0 and not tf_mask[t - 1]:
                        nc.vector.tensor_copy(
                            xk[0:1, s, :],
                            outbuf[0:1, ((t - 1) % 8) * B:((t - 1) % 8 + 1) * B])

                    # ---- layer 0
                    sig0 = []
                    for X in range(4):
                        g0 = gpsum.tile([128, LJ, B], F32, tag="g",
                                        name=f"g0_{_rep}_{t}_{X}")
                        for k in range(1, NJ + 1):
                            for j in range(LJ):
                                nc.tensor.matmul(
                                    g0[:, j, :], w0_sl(k, X, j),
                                    h0f[:, k - 1, :],
                                    start=(k == 1), stop=False)
                        for j in range(LJ):
                            base = (X * LJ + j) * 128
                            nc.tensor.matmul(
                                g0[:, j, :], w0_sb[0:KX, base:base + 128],
                                xk[:, s, :], start=False, stop=True)
                        sX = actp.tile([128, LJ, B], BF16, tag=f"s{X}",
                                       name=f"s0_{_rep}_{t}_{X}")
                        nc.scalar.activation(sX[:], g0[:], AFS[X])
                        sig0.append(sX)
                    hh0 = cell(sig0, c0, "hh0", t, "l0")
                    gather(hh0, h0f, "0", f"{_rep}_{t}")

                    # ---- layer 1
                    sig1 = []
                    for X in range(4):
                        g1 = gpsum.tile([128, LJ, B], F32, tag="g",
                                        name=f"g1_{_rep}_{t}_{X}")
                        for k in range(NJ):
                            for j in range(LJ):
                                nc.tensor.matmul(
                                    g1[:, j, :], w1_sl(k, X, j), h1f[:, k, :],
                                    start=(k == 0), stop=False)
                        for k in range(NJ):
                            for j in range(LJ):
                                nc.tensor.matmul(
                                    g1[:, j, :], w1_sl(8 + k, X, j),
                                    h0f[:, k, :],
                                    start=False, stop=(k == NJ - 1))
                        sX = actp.tile([128, LJ, B], BF16, tag=f"s{X}",
                                       name=f"s1_{_rep}_{t}_{X}")
                        if B1_BCAST[0]:
                            try:
                                bb = b1s_sb[:, X * LJ:(X + 1) * LJ] \
                                    .broadcast_to((128, LJ, B))
                                nc.vector.tensor_tensor(
                                    out=g1[:], in0=g1[:], in1=bb, op=ALU.add)
                                nc.scalar.activation(sX[:], g1[:], AFS[X])
                            except Exception:
                                B1_BCAST[0] = False
                        if not B1_BCAST[0]:
                            for j in range(LJ):
                                nc.scalar.activation(
                                    sX[:, j, :], g1[:, j, :], AFS[X],
                                    bias=b1s_sb[:, X * LJ + j:X * LJ + j + 1])
                        sig1.append(sX)
                    hh1 = cell(sig1, c1, "hh1", t, "l1")
                    gather(hh1, h1f, "1", f"{_rep}_{t}")

                emit_pred(T - 1)
                nc.sync.dma_start(out=out_d[NB8 - 1:NB8, :], in_=outbuf[:])

            for _r in range(rep):
                rep_body(_r)

    nc.finalize()
    return nc


def kernel(**inputs):
    import time
    from concourse.bass_utils import run_bass_kernel_spmd
    T = T_FULL
    shared, per_core, tf_mask, b_proj = prep_host(inputs, T)
    nc = build_module(T, tf_mask, b_proj)
    in_maps = []
    for c in range(N_CORES):
        m = dict(shared)
        m.update(per_core[c])
        in_maps.append(m)
    res = None
    for attempt in range(3):
        try:
            res = run_bass_kernel_spmd(nc, in_maps, list(range(N_CORES)))
            break
        except Exception:
            if attempt == 2:
                raise
            time.sleep(5)
    ob = res.results[0]["out"].astype(np.float32)      # (NB8, 8*B)
    out = np.zeros((B, T, 1), np.float32)
    for t in range(T):
        b8, s = divmod(t, 8)
        out[:, t, 0] = ob[b8, s * B:(s + 1) * B]
    return out



# revision 2
# speedup vs baseline: 173.3300x; 173.3300x over previous
"""2-layer LSTM decoder for trn2 — single-core, hardware-looped.

The execution backend (axon/fake_nrt) charges wall time per STATIC
instruction (~55us each); dynamic re-executions inside hardware For_i
loops are ~free, cores emulate in parallel, and instruction size barely
matters.  So the whole T=168 recurrence runs on ONE core inside a
For_i(0,T) loop whose body is static (~150 instructions total):

- Weights live in DRAM, laid out per hidden-chunk j; each j-iteration
  of an inner For_i(0,8) DMAs the [128, 4*K*128] slice for all 4 gates
  into a fixed SBUF tile (dynamic DRAM APs are allowed; matmul
  stationary APs must be static, so the dynamic index rides the DMA).
- Gate psums [128gate x 512batch] accumulate over K-chunks; activations
  write sig tiles at the dynamic j offset (allowed on act outputs).
- L1's bias is a 17th K-chunk: stationary [1,128] b1 slice against a
  constant ones [1,512] row.  L0's bias rides the xk ones row.
- Teacher forcing is data-driven: host precomputes knb row0 (y*tf) and
  tfc (1-tf); in-loop, xk_row0 += tfc[t] * pred(t-1), so no per-step
  branching and the loop body stays static.
"""
import numpy as np
import ml_dtypes

import concourse.bass as bass
import concourse.mybir as mybir
import concourse.tile as tile
from concourse import bacc

F32 = mybir.dt.float32
BF16 = mybir.dt.bfloat16
AF = mybir.ActivationFunctionType
ALU = mybir.AluOpType

B, T_FULL, F, H, GE = 512, 168, 32, 1024, 16
N_CORES = 1
NJ = 8                     # hidden chunks (H/128)
KX = 50                    # xk rows: prev_y(1) + known(32) + gv(16) + ones(1)
K0 = 9                     # L0 K-chunks per gate: 8 h + 1 xk
K1 = 17                    # L1 K-chunks per gate: 8 h1 + 8 h0new + 1 bias


def prep_host(inputs, T):
    inp = {k: np.asarray(v) for k, v in inputs.items()}
    gv = inp["group_emb"][inp["group_ids"].astype(np.int64)]       # (B, GE)
    b0 = (inp["b_ih0"] + inp["b_hh0"]).astype(np.float32)          # (4096,)
    b1 = (inp["b_ih1"] + inp["b_hh1"]).astype(np.float32)
    Whh0 = inp["W_hh0"].astype(np.float32)                         # (4096, 1024)
    Whh1 = inp["W_hh1"].astype(np.float32)
    Wih1 = inp["W_ih1"].astype(np.float32)
    Wih0 = inp["W_ih0"].astype(np.float32)                         # (4096, 49)

    # w0[j, p, (X*9+k)*128+m]: k<8 -> Whh0[X*1024+j*128+m, k*128+p],
    #                          k=8 -> rows 0:49 Wih0, row 49 b0.
    A = np.zeros((NJ, 128, 4, K0, 128), np.float32)
    A[:, :, :, :8, :] = Whh0.reshape(4, NJ, 128, 8, 128).transpose(1, 4, 0, 3, 2)
    A[:, :49, :, 8, :] = Wih0.reshape(4, NJ, 128, 49).transpose(1, 3, 0, 2)
    A[:, 49, :, 8, :] = b0.reshape(4, NJ, 128).transpose(1, 0, 2)
    w0 = np.ascontiguousarray(A.reshape(NJ, 128, 4 * K0 * 128)).astype(
        ml_dtypes.bfloat16)

    # w1[j, p, (X*17+k)*128+m]: k<8 Whh1, k in 8..15 Wih1, k=16 row0 b1.
    Bm = np.zeros((NJ, 128, 4, K1, 128), np.float32)
    Bm[:, :, :, :8, :] = Whh1.reshape(4, NJ, 128, 8, 128).transpose(1, 4, 0, 3, 2)
    Bm[:, :, :, 8:16, :] = Wih1.reshape(4, NJ, 128, 8, 128).transpose(1, 4, 0, 3, 2)
    Bm[:, 0, :, 16, :] = b1.reshape(4, NJ, 128).transpose(1, 0, 2)
    w1 = np.ascontiguousarray(Bm.reshape(NJ, 128, 4 * K1 * 128)).astype(
        ml_dtypes.bfloat16)

    wp = inp["W_proj"].astype(np.float32)[0]                       # (1024,)
    wpT = np.ascontiguousarray(wp.reshape(NJ, 128).T).astype(ml_dtypes.bfloat16)

    y = inp["target_y"].astype(np.float32)[:, :, 0]                # (B, T)
    tf = np.asarray(inp["tf_mask"]).reshape(-1).astype(np.float32)[:T]
    knb = np.zeros((T, KX, B), np.float32)
    knb[0, 0] = inp["last_enc_consumption"].astype(np.float32)[:, 0]
    for t in range(1, T):
        knb[t, 0] = tf[t - 1] * y[:, t - 1]
    knb[:, 1:33] = inp["dec_known"].astype(np.float32)[:, :T, :].transpose(1, 2, 0)
    knb[:, 33:49] = gv.T[None]
    knb[:, 49] = 1.0
    knb = knb.astype(ml_dtypes.bfloat16)

    tfc = np.zeros((T, 1, B), np.float32)
    for t in range(1, T):
        tfc[t, 0] = 1.0 - tf[t - 1]
    tfc = tfc.astype(ml_dtypes.bfloat16)

    def st(a):          # (B, H) -> [128, NJ, B]
        return np.ascontiguousarray(
            a.astype(np.float32).reshape(B, NJ, 128).transpose(2, 1, 0))

    shared = dict(
        w0=w0, w1=w1, wpT=wpT, knb=knb, tfc=tfc,
        h0i=st(inp["h0"][0]).astype(ml_dtypes.bfloat16),
        h1i=st(inp["h0"][1]).astype(ml_dtypes.bfloat16),
        c0i=st(inp["c0"][0]),
        c1i=st(inp["c0"][1]),
    )
    per_core = [dict() for _ in range(N_CORES)]
    tf_mask = [int(v) for v in np.asarray(inp["tf_mask"]).reshape(-1)][:T]
    b_proj = float(np.asarray(inp["b_proj"]).reshape(-1)[0])
    return shared, per_core, tf_mask, b_proj


def build_module(T, tf_mask, b_proj, rep=1):
    nc = bacc.Bacc(target_bir_lowering=False)

    w0_d = nc.dram_tensor("w0", [NJ, 128, 4 * K0 * 128], BF16, kind="ExternalInput")
    w1_d = nc.dram_tensor("w1", [NJ, 128, 4 * K1 * 128], BF16, kind="ExternalInput")
    wpT_d = nc.dram_tensor("wpT", [128, NJ], BF16, kind="ExternalInput")
    knb_d = nc.dram_tensor("knb", [T, KX, B], BF16, kind="ExternalInput")
    tfc_d = nc.dram_tensor("tfc", [T, 1, B], BF16, kind="ExternalInput")
    h0i_d = nc.dram_tensor("h0i", [128, NJ, B], BF16, kind="ExternalInput")
    h1i_d = nc.dram_tensor("h1i", [128, NJ, B], BF16, kind="ExternalInput")
    c0i_d = nc.dram_tensor("c0i", [128, NJ, B], F32, kind="ExternalInput")
    c1i_d = nc.dram_tensor("c1i", [128, NJ, B], F32, kind="ExternalInput")
    out_d = nc.dram_tensor("out", [T, 1, B], F32, kind="ExternalOutput")

    AFS = [AF.Sigmoid, AF.Sigmoid, AF.Tanh, AF.Sigmoid]   # i, f, g, o

    with tile.TileContext(nc) as tc:
        with tc.tile_pool(name="const", bufs=1) as const, \
             tc.tile_pool(name="state", bufs=1) as stp, \
             tc.tile_pool(name="act", bufs=1) as actp, \
             tc.tile_pool(name="wld", bufs=2) as wld, \
             tc.tile_pool(name="io", bufs=2) as iop, \
             tc.tile_pool(name="gps", bufs=1, space="PSUM") as gpsum:

            wpT = const.tile([128, NJ], BF16)
            nc.sync.dma_start(out=wpT[:], in_=wpT_d[:])
            ones = const.tile([1, B], BF16)
            nc.vector.memset(ones[:], 1.0)

            def rep_body(r):
                h0f = stp.tile([128, NJ, B], BF16, tag="h0f", name=f"h0_{r}")
                nc.sync.dma_start(out=h0f[:], in_=h0i_d[:])
                h1f = stp.tile([128, NJ, B], BF16, tag="h1f", name=f"h1_{r}")
                nc.sync.dma_start(out=h1f[:], in_=h1i_d[:])
                c0 = stp.tile([128, NJ, B], F32, tag="c0", name=f"c0_{r}")
                nc.sync.dma_start(out=c0[:], in_=c0i_d[:])
                c1 = stp.tile([128, NJ, B], F32, tag="c1", name=f"c1_{r}")
                nc.sync.dma_start(out=c1[:], in_=c1i_d[:])
                pred = stp.tile([1, B], F32, tag="pred", name=f"pred_{r}")
                nc.vector.memset(pred[:], 0.0)

                sig = [actp.tile([128, NJ, B], BF16, tag=f"sig{X}",
                                 name=f"sig{X}_{r}") for X in range(4)]

                def cell(c_cur, hf, lab):
                    tmpf = actp.tile([128, NJ, B], F32, tag="tmpf",
                                     name=f"tf_{lab}")
                    nc.vector.tensor_tensor(out=tmpf[:], in0=sig[1][:],
                                            in1=c_cur[:], op=ALU.mult)
                    tmpb = actp.tile([128, NJ, B], BF16, tag="tmpb",
                                     name=f"tb_{lab}")
                    nc.vector.tensor_tensor(out=tmpb[:], in0=sig[0][:],
                                            in1=sig[2][:], op=ALU.mult)
                    nc.vector.tensor_tensor(out=c_cur[:], in0=tmpf[:],
                                            in1=tmpb[:], op=ALU.add)
                    tanc = actp.tile([128, NJ, B], BF16, tag="tanc",
                                     name=f"tc_{lab}")
                    nc.scalar.activation(tanc[:], c_cur[:], AF.Tanh)
                    nc.vector.tensor_tensor(out=hf[:], in0=sig[3][:],
                                            in1=tanc[:], op=ALU.mult)

                with tc.For_i(0, T) as it:
                    # ---- assemble x(t): load block, add tfc*pred into row 0
                    xk = iop.tile([KX, B], BF16, tag="xk")
                    nc.sync.dma_start(out=xk[:], in_=knb_d[it])
                    tfr = iop.tile([1, B], BF16, tag="tfr")
                    nc.sync.dma_start(out=tfr[:], in_=tfc_d[it])
                    fb = iop.tile([1, B], BF16, tag="fb")
                    nc.vector.tensor_tensor(out=fb[:], in0=pred[:],
                                            in1=tfr[:], op=ALU.mult)
                    nc.vector.tensor_tensor(out=xk[0:1, :], in0=xk[0:1, :],
                                            in1=fb[:], op=ALU.add)

                    # ---- layer 0: gates for hidden chunk j
                    with tc.For_i(0, NJ) as jv:
                        w0c = wld.tile([128, 4 * K0 * 128], BF16, tag="w0c")
                        nc.sync.dma_start(out=w0c[:], in_=w0_d[jv])
                        for X in range(4):
                            g = gpsum.tile([128, B], F32, tag=f"g{X}")
                            for k in range(8):
                                nc.tensor.matmul(
                                    g[:], w0c[:, (X * K0 + k) * 128:
                                              (X * K0 + k + 1) * 128],
                                    h0f[:, k, :], start=(k == 0), stop=False)
                            nc.tensor.matmul(
                                g[:], w0c[0:KX, (X * K0 + 8) * 128:
                                          (X * K0 + 9) * 128],
                                xk[:], start=False, stop=True)
                            nc.scalar.activation(sig[X][:, jv], g[:], AFS[X])
                    cell(c0, h0f, "l0")

                    # ---- layer 1
                    with tc.For_i(0, NJ) as jv:
                        w1c = wld.tile([128, 4 * K1 * 128], BF16, tag="w1c")
                        nc.sync.dma_start(out=w1c[:], in_=w1_d[jv])
                        for X in range(4):
                            g = gpsum.tile([128, B], F32, tag=f"g{X}")
                            for k in range(8):
                                nc.tensor.matmul(
                                    g[:], w1c[:, (X * K1 + k) * 128:
                                              (X * K1 + k + 1) * 128],
                                    h1f[:, k, :], start=(k == 0), stop=False)
                            for k in range(8, 16):
                                nc.tensor.matmul(
                                    g[:], w1c[:, (X * K1 + k) * 128:
                                              (X * K1 + k + 1) * 128],
                                    h0f[:, k - 8, :], start=False, stop=False)
                            nc.tensor.matmul(
                                g[:], w1c[0:1, (X * K1 + 16) * 128:
                                          (X * K1 + 17) * 128],
                                ones[:], start=False, stop=True)
                            nc.scalar.activation(sig[X][:, jv], g[:], AFS[X])
                    cell(c1, h1f, "l1")

                    # ---- pred(t) = wp . h1 + b_proj
                    pp = gpsum.tile([1, B], F32, tag="pp")
                    for k in range(NJ):
                        nc.tensor.matmul(pp[:], wpT[:, k:k + 1], h1f[:, k, :],
                                         start=(k == 0), stop=(k == NJ - 1))
                    nc.vector.tensor_scalar_add(pred[:], pp[:], b_proj)
                    nc.sync.dma_start(out=out_d[it], in_=pred[:])

            for r in range(rep):
                rep_body(r)

    nc.finalize()
    return nc


def kernel(**inputs):
    import time
    from concourse.bass_utils import run_bass_kernel_spmd
    T = T_FULL
    shared, per_core, tf_mask, b_proj = prep_host(inputs, T)
    nc = build_module(T, tf_mask, b_proj)
    in_maps = []
    for c in range(N_CORES):
        m = dict(shared)
        m.update(per_core[c])
        in_maps.append(m)
    res = None
    for attempt in range(3):
        try:
            res = run_bass_kernel_spmd(nc, in_maps, list(range(N_CORES)))
            break
        except Exception:
            if attempt == 2:
                raise
            time.sleep(5)
    ob = res.results[0]["out"].astype(np.float32)      # (T, 1, B)
    return np.ascontiguousarray(ob[:, 0, :].T)[:, :, None]  # (B, T, 1)


# revision 3
# speedup vs baseline: 895.9257x; 5.1689x over previous
"""2-layer LSTM decoder for trn2 — single-core, hardware-looped.

The execution backend (axon/fake_nrt) charges wall time per STATIC
instruction (~55us each); dynamic re-executions inside hardware For_i
loops are ~free, cores emulate in parallel, and instruction size barely
matters.  So the whole T=168 recurrence runs on ONE core inside a
For_i(0,T) loop whose body is static (~150 instructions total):

- Weights live in DRAM, laid out per hidden-chunk j; each j-iteration
  of an inner For_i(0,8) DMAs the [128, 4*K*128] slice for all 4 gates
  into a fixed SBUF tile (dynamic DRAM APs are allowed; matmul
  stationary APs must be static, so the dynamic index rides the DMA).
- Gate psums [128gate x 512batch] accumulate over K-chunks; activations
  write sig tiles at the dynamic j offset (allowed on act outputs).
- L1's bias is a 17th K-chunk: stationary [1,128] b1 slice against a
  constant ones [1,512] row.  L0's bias rides the xk ones row.
- Teacher forcing is data-driven: host precomputes knb row0 (y*tf) and
  tfc (1-tf); in-loop, xk_row0 += tfc[t] * pred(t-1), so no per-step
  branching and the loop body stays static.
"""
import numpy as np
import ml_dtypes

import concourse.bass as bass
import concourse.mybir as mybir
import concourse.tile as tile
from concourse import bacc

F32 = mybir.dt.float32
BF16 = mybir.dt.bfloat16
AF = mybir.ActivationFunctionType
ALU = mybir.AluOpType

B, T_FULL, F, H, GE = 512, 168, 32, 1024, 16
N_CORES = 1
NJ = 8                     # hidden chunks (H/128)
KX = 50                    # xk rows: prev_y(1) + known(32) + gv(16) + ones(1)
K0 = 9                     # L0 K-chunks per gate: 8 h + 1 xk
K1 = 17                    # L1 K-chunks per gate: 8 h1 + 8 h0new + 1 bias


def prep_host(inputs, T):
    inp = {k: np.asarray(v) for k, v in inputs.items()}
    gv = inp["group_emb"][inp["group_ids"].astype(np.int64)]       # (B, GE)
    b0 = (inp["b_ih0"] + inp["b_hh0"]).astype(np.float32)          # (4096,)
    b1 = (inp["b_ih1"] + inp["b_hh1"]).astype(np.float32)
    Whh0 = inp["W_hh0"].astype(np.float32)                         # (4096, 1024)
    Whh1 = inp["W_hh1"].astype(np.float32)
    Wih1 = inp["W_ih1"].astype(np.float32)
    Wih0 = inp["W_ih0"].astype(np.float32)                         # (4096, 49)

    # w0[j, p, (X*9+k)*128+m]: k<8 -> Whh0[X*1024+j*128+m, k*128+p],
    #                          k=8 -> rows 0:49 Wih0, row 49 b0.
    A = np.zeros((NJ, 128, 4, K0, 128), np.float32)
    A[:, :, :, :8, :] = Whh0.reshape(4, NJ, 128, 8, 128).transpose(1, 4, 0, 3, 2)
    A[:, :49, :, 8, :] = Wih0.reshape(4, NJ, 128, 49).transpose(1, 3, 0, 2)
    A[:, 49, :, 8, :] = b0.reshape(4, NJ, 128).transpose(1, 0, 2)
    w0 = np.ascontiguousarray(A.reshape(NJ, 128, 4 * K0 * 128)).astype(
        ml_dtypes.bfloat16)

    # w1[j, p, (X*17+k)*128+m]: k<8 Whh1, k in 8..15 Wih1, k=16 row0 b1.
    Bm = np.zeros((NJ, 128, 4, K1, 128), np.float32)
    Bm[:, :, :, :8, :] = Whh1.reshape(4, NJ, 128, 8, 128).transpose(1, 4, 0, 3, 2)
    Bm[:, :, :, 8:16, :] = Wih1.reshape(4, NJ, 128, 8, 128).transpose(1, 4, 0, 3, 2)
    Bm[:, 0, :, 16, :] = b1.reshape(4, NJ, 128).transpose(1, 0, 2)
    w1 = np.ascontiguousarray(Bm.reshape(NJ, 128, 4 * K1 * 128)).astype(
        ml_dtypes.bfloat16)

    wp = inp["W_proj"].astype(np.float32)[0]                       # (1024,)
    wpT = np.ascontiguousarray(wp.reshape(NJ, 128).T).astype(ml_dtypes.bfloat16)

    y = inp["target_y"].astype(np.float32)[:, :, 0]                # (B, T)
    tf = np.asarray(inp["tf_mask"]).reshape(-1).astype(np.float32)[:T]
    knb = np.zeros((T, KX, B), np.float32)
    knb[0, 0] = inp["last_enc_consumption"].astype(np.float32)[:, 0]
    for t in range(1, T):
        knb[t, 0] = tf[t - 1] * y[:, t - 1]
    knb[:, 1:33] = inp["dec_known"].astype(np.float32)[:, :T, :].transpose(1, 2, 0)
    knb[:, 33:49] = gv.T[None]
    knb[:, 49] = 1.0
    knb = knb.astype(ml_dtypes.bfloat16)

    tfc = np.zeros((T, 1, B), np.float32)
    for t in range(1, T):
        tfc[t, 0] = 1.0 - tf[t - 1]
    tfc = tfc.astype(ml_dtypes.bfloat16)

    def st(a):          # (B, H) -> [128, NJ, B]
        return np.ascontiguousarray(
            a.astype(np.float32).reshape(B, NJ, 128).transpose(2, 1, 0))

    shared = dict(
        w0=w0, w1=w1, wpT=wpT, knb=knb, tfc=tfc,
        h0i=st(inp["h0"][0]).astype(ml_dtypes.bfloat16),
        h1i=st(inp["h0"][1]).astype(ml_dtypes.bfloat16),
        c0i=st(inp["c0"][0]),
        c1i=st(inp["c0"][1]),
    )
    per_core = [dict() for _ in range(N_CORES)]
    tf_mask = [int(v) for v in np.asarray(inp["tf_mask"]).reshape(-1)][:T]
    b_proj = float(np.asarray(inp["b_proj"]).reshape(-1)[0])
    return shared, per_core, tf_mask, b_proj


def build_module(T, tf_mask, b_proj, rep=1):
    nc = bacc.Bacc(target_bir_lowering=False)

    w0_d = nc.dram_tensor("w0", [NJ, 128, 4 * K0 * 128], BF16, kind="ExternalInput")
    w1_d = nc.dram_tensor("w1", [NJ, 128, 4 * K1 * 128], BF16, kind="ExternalInput")
    wpT_d = nc.dram_tensor("wpT", [128, NJ], BF16, kind="ExternalInput")
    knb_d = nc.dram_tensor("knb", [T, KX, B], BF16, kind="ExternalInput")
    tfc_d = nc.dram_tensor("tfc", [T, 1, B], BF16, kind="ExternalInput")
    h0i_d = nc.dram_tensor("h0i", [128, NJ, B], BF16, kind="ExternalInput")
    h1i_d = nc.dram_tensor("h1i", [128, NJ, B], BF16, kind="ExternalInput")
    c0i_d = nc.dram_tensor("c0i", [128, NJ, B], F32, kind="ExternalInput")
    c1i_d = nc.dram_tensor("c1i", [128, NJ, B], F32, kind="ExternalInput")
    out_d = nc.dram_tensor("out", [T, 1, B], F32, kind="ExternalOutput")

    AFS = [AF.Sigmoid, AF.Sigmoid, AF.Tanh, AF.Sigmoid]   # i, f, g, o

    with tile.TileContext(nc) as tc:
        with tc.tile_pool(name="const", bufs=1) as const, \
             tc.tile_pool(name="state", bufs=1) as stp, \
             tc.tile_pool(name="act", bufs=1) as actp, \
             tc.tile_pool(name="wld", bufs=1) as wld, \
             tc.tile_pool(name="io", bufs=2) as iop, \
             tc.tile_pool(name="gps", bufs=1, space="PSUM") as gpsum:

            wpT = const.tile([128, NJ], BF16)
            nc.sync.dma_start(out=wpT[:], in_=wpT_d[:])
            ones = const.tile([1, B], BF16)
            nc.vector.memset(ones[:], 1.0)

            def rep_body(r):
                h0f = stp.tile([128, NJ, B], BF16, tag="h0f", name=f"h0_{r}")
                nc.sync.dma_start(out=h0f[:], in_=h0i_d[:])
                h1f = stp.tile([128, NJ, B], BF16, tag="h1f", name=f"h1_{r}")
                nc.sync.dma_start(out=h1f[:], in_=h1i_d[:])
                c0 = stp.tile([128, NJ, B], F32, tag="c0", name=f"c0_{r}")
                nc.sync.dma_start(out=c0[:], in_=c0i_d[:])
                c1 = stp.tile([128, NJ, B], F32, tag="c1", name=f"c1_{r}")
                nc.sync.dma_start(out=c1[:], in_=c1i_d[:])
                pred = stp.tile([1, B], F32, tag="pred", name=f"pred_{r}")
                nc.vector.memset(pred[:], 0.0)

                sig = [actp.tile([128, NJ, B], BF16, tag=f"sig{X}",
                                 name=f"sig{X}_{r}") for X in range(4)]

                def cell(c_cur, hf, lab):
                    tmpf = actp.tile([128, NJ, B], F32, tag="tmpf",
                                     name=f"tf_{lab}")
                    nc.vector.tensor_tensor(out=tmpf[:], in0=sig[1][:],
                                            in1=c_cur[:], op=ALU.mult)
                    tmpb = actp.tile([128, NJ, B], BF16, tag="tmpb",
                                     name=f"tb_{lab}")
                    nc.vector.tensor_tensor(out=tmpb[:], in0=sig[0][:],
                                            in1=sig[2][:], op=ALU.mult)
                    nc.vector.tensor_tensor(out=c_cur[:], in0=tmpf[:],
                                            in1=tmpb[:], op=ALU.add)
                    tanc = actp.tile([128, NJ, B], BF16, tag="tanc",
                                     name=f"tc_{lab}")
                    nc.scalar.activation(tanc[:], c_cur[:], AF.Tanh)
                    nc.vector.tensor_tensor(out=hf[:], in0=sig[3][:],
                                            in1=tanc[:], op=ALU.mult)

                with tc.For_i(0, T) as it:
                    # ---- assemble x(t): load block, add tfc*pred into row 0
                    xk = iop.tile([KX, B], BF16, tag="xk")
                    nc.sync.dma_start(out=xk[:], in_=knb_d[it])
                    tfr = iop.tile([1, B], BF16, tag="tfr")
                    nc.sync.dma_start(out=tfr[:], in_=tfc_d[it])
                    fb = iop.tile([1, B], BF16, tag="fb")
                    nc.vector.tensor_tensor(out=fb[:], in0=pred[:],
                                            in1=tfr[:], op=ALU.mult)
                    nc.vector.tensor_tensor(out=xk[0:1, :], in0=xk[0:1, :],
                                            in1=fb[:], op=ALU.add)

                    # ---- layer 0: gates for hidden chunks (jv, jv+1)
                    with tc.For_i(0, NJ, 2) as jv:
                        w0a = wld.tile([128, 4 * K0 * 128], BF16, tag="w0a")
                        nc.sync.dma_start(out=w0a[:], in_=w0_d[jv])
                        w0b = wld.tile([128, 4 * K0 * 128], BF16, tag="w0b")
                        nc.sync.dma_start(out=w0b[:], in_=w0_d[jv + 1])
                        for s, w0c in ((0, w0a), (1, w0b)):
                            for X in range(4):
                                g = gpsum.tile([128, B], F32, tag=f"g{X}")
                                for k in range(8):
                                    nc.tensor.matmul(
                                        g[:], w0c[:, (X * K0 + k) * 128:
                                                  (X * K0 + k + 1) * 128],
                                        h0f[:, k, :], start=(k == 0), stop=False)
                                nc.tensor.matmul(
                                    g[:], w0c[0:KX, (X * K0 + 8) * 128:
                                              (X * K0 + 9) * 128],
                                    xk[:], start=False, stop=True)
                                nc.scalar.activation(sig[X][:, jv + s], g[:],
                                                     AFS[X])
                    cell(c0, h0f, "l0")

                    # ---- layer 1
                    with tc.For_i(0, NJ, 2) as jv:
                        w1a = wld.tile([128, 4 * K1 * 128], BF16, tag="w1a")
                        nc.sync.dma_start(out=w1a[:], in_=w1_d[jv])
                        w1b = wld.tile([128, 4 * K1 * 128], BF16, tag="w1b")
                        nc.sync.dma_start(out=w1b[:], in_=w1_d[jv + 1])
                        for s, w1c in ((0, w1a), (1, w1b)):
                            for X in range(4):
                                g = gpsum.tile([128, B], F32, tag=f"g{X}")
                                for k in range(8):
                                    nc.tensor.matmul(
                                        g[:], w1c[:, (X * K1 + k) * 128:
                                                  (X * K1 + k + 1) * 128],
                                        h1f[:, k, :], start=(k == 0), stop=False)
                                for k in range(8, 16):
                                    nc.tensor.matmul(
                                        g[:], w1c[:, (X * K1 + k) * 128:
                                                  (X * K1 + k + 1) * 128],
                                        h0f[:, k - 8, :], start=False, stop=False)
                                nc.tensor.matmul(
                                    g[:], w1c[0:1, (X * K1 + 16) * 128:
                                              (X * K1 + 17) * 128],
                                    ones[:], start=False, stop=True)
                                nc.scalar.activation(sig[X][:, jv + s], g[:],
                                                     AFS[X])
                    cell(c1, h1f, "l1")

                    # ---- pred(t) = wp . h1 + b_proj
                    pp = gpsum.tile([1, B], F32, tag="pp")
                    for k in range(NJ):
                        nc.tensor.matmul(pp[:], wpT[:, k:k + 1], h1f[:, k, :],
                                         start=(k == 0), stop=(k == NJ - 1))
                    nc.vector.tensor_scalar_add(pred[:], pp[:], b_proj)
                    nc.sync.dma_start(out=out_d[it], in_=pred[:])

            for r in range(rep):
                rep_body(r)

    nc.finalize()
    return nc


def kernel(**inputs):
    import time
    from concourse.bass_utils import run_bass_kernel_spmd
    T = T_FULL
    shared, per_core, tf_mask, b_proj = prep_host(inputs, T)
    nc = build_module(T, tf_mask, b_proj)
    in_maps = []
    for c in range(N_CORES):
        m = dict(shared)
        m.update(per_core[c])
        in_maps.append(m)
    res = None
    for attempt in range(3):
        try:
            res = run_bass_kernel_spmd(nc, in_maps, list(range(N_CORES)))
            break
        except Exception:
            if attempt == 2:
                raise
            time.sleep(5)
    ob = res.results[0]["out"].astype(np.float32)      # (T, 1, B)
    return np.ascontiguousarray(ob[:, 0, :].T)[:, :, None]  # (B, T, 1)
